# revision 1
# baseline (speedup 1.0000x reference)
"""Mixtral GQA attention block (B=1, S=2048, HID=4096, NH=32, NKV=8, HD=128),
8-way tensor-parallel over heads on trn2: each core owns 4 query heads + 1 KV
head (one GQA group), computes its partial output projection, host sums the
8 partials.

Device layout notes:
  - All matmul operands are staged transposed (contraction dim on partitions).
    Host pre-packs every tensor partition-major so DMAs are identity copies.
  - Scores are computed transposed (S^T[k,q]) so the exp'd tiles directly
    serve as lhsT for the P@V matmul; softmax denominator comes from an
    appended ones-column on V; no max-subtraction (bf16 probabilities cannot
    overflow; scores are O(20)).
  - Causality: only k-tiles j with j*128 <= q_max are computed; the 4
    diagonal-band blocks use precomputed 0/1 multiplicative masks.
  - RoPE is applied in the transposed orientation; the half-swap crosses
    partitions so it runs as a PE matmul against a constant 128x128
    half-rotation permutation matrix.
"""

import math
import os
import sys

import numpy as np

sys.path.insert(0, "/opt/trn_rl_repo")

import concourse.bass as bass
import concourse.tile as tile
from concourse import bacc
from concourse import mybir

S = 2048
HID = 4096
NH, NKV, HD = 32, 8, 128
NCORES = 8
QH = NH // NCORES      # 4 query heads per core
ND = HID // 128        # 32 contraction chunks
NI = S // 512          # 4 q-chunks of 512
NJ = S // 128          # 16 k-tiles of 128
SCALE = 1.0 / math.sqrt(HD)

F16 = mybir.dt.float16
BF16 = mybir.dt.bfloat16
F32 = mybir.dt.float32

_CACHE = {}
LAST_RESULTS = None


def _build_program():
    nc = bacc.Bacc()

    ht = nc.declare_dram_parameter("ht", [128, ND, S], F16, isOutput=False)
    wq = nc.declare_dram_parameter("wq", [128, ND, 512], F16, isOutput=False)
    wk = nc.declare_dram_parameter("wk", [128, ND, 128], F16, isOutput=False)
    wv = nc.declare_dram_parameter("wv", [128, ND, 128], F16, isOutput=False)
    wo = nc.declare_dram_parameter("wo", [128, QH, HID], F16, isOutput=False)
    cosd = nc.declare_dram_parameter("cosd", [128, S], F16, isOutput=False)
    pswap = nc.declare_dram_parameter("pswap", [128, 128], F16, isOutput=False)
    identd = nc.declare_dram_parameter("identd", [128, 128], F16, isOutput=False)
    identb = nc.declare_dram_parameter("identb", [128, 128], BF16, isOutput=False)
    maskd = nc.declare_dram_parameter("maskd", [128, 4, 512], BF16, isOutput=False)
    sind = nc.declare_dram_parameter("sind", [128, S], F16, isOutput=False)
    out = nc.declare_dram_parameter("out", [S, HID], F16, isOutput=True)

    with tile.TileContext(nc) as tc:
        with (
            tc.tile_pool(name="consts", bufs=1) as consts,
            tc.tile_pool(name="hpool", bufs=10) as hpool,
            tc.tile_pool(name="ptpool", bufs=5) as ptpool,
            tc.tile_pool(name="rtmp", bufs=3) as rtmp,
            tc.tile_pool(name="small", bufs=8) as small,
            tc.tile_pool(name="orow", bufs=5) as orowp,
            tc.tile_pool(name="pproj", bufs=2, space="PSUM") as pproj,
            tc.tile_pool(name="pwork", bufs=2, space="PSUM") as pwork,
            tc.tile_pool(name="popsum", bufs=4, space="PSUM") as popsum,
        ):
            # First pass activations + first head weights are the startup
            # critical path: enqueue exactly what the first matmuls touch
            # before any bulk weight traffic.
            wq_sb = consts.tile([128, ND, 512], F16)
            cos_sb = consts.tile([128, S], F16)
            sin_sb = consts.tile([128, S], F16)
            pswap_sb = consts.tile([128, 128], F16)
            wk_sb = consts.tile([128, ND, 128], F16)
            wv_sb = consts.tile([128, ND, 128], F16)
            wo_sb = consts.tile([128, QH, HID], F16)
            ident16 = consts.tile([128, 128], F16)
            identbf = consts.tile([128, 128], BF16)
            masks = consts.tile([128, 4, 512], BF16)

            hts0 = []
            for dq in range(8):
                t_h = hpool.tile([128, 4, 512], F16, tag="ht",
                                 name=f"ht_0_{dq}")
                hts0.append(t_h)

            def ht_dma(tiles, I, dq):
                nc.sync.dma_start(
                    out=tiles[dq],
                    in_=ht[:, dq * 4:(dq + 1) * 4, I * 512:(I + 1) * 512],
                )

            nc.sync.dma_start(out=wk_sb, in_=wk[:, :, :])
            for dq in range(8):
                ht_dma(hts0, 0, dq)
            nc.sync.dma_start(out=wv_sb, in_=wv[:, :, :])
            nc.sync.dma_start(out=cos_sb, in_=cosd[:, :])
            nc.sync.dma_start(out=sin_sb, in_=sind[:, :])
            nc.sync.dma_start(out=pswap_sb, in_=pswap[:, :])
            nc.sync.dma_start(out=identbf, in_=identb[:, :])
            # 0/1 causal masks for the 4 diagonal-band block offsets
            # (host-built): mask[m][p, f] = 1.0 iff f - p - 128*m >= 0
            nc.sync.dma_start(out=masks, in_=maskd[:, :, :])
            for tt in range(QH):
                for dh in range(2):
                    nc.sync.dma_start(
                        out=wq_sb[:, dh * 16:(dh + 1) * 16,
                                  tt * 128:(tt + 1) * 128],
                        in_=wq[:, dh * 16:(dh + 1) * 16,
                               tt * 128:(tt + 1) * 128],
                    )
            nc.sync.dma_start(out=ident16, in_=identd[:, :])
            for oc in range(QH):
                for dh in range(2):
                    nc.sync.dma_start(
                        out=wo_sb[:, oc, dh * 2048:(dh + 1) * 2048],
                        in_=wo[:, oc, dh * 2048:(dh + 1) * 2048],
                    )

            qT = consts.tile([128, QH, S], F16)    # roped q, transposed
            kT = consts.tile([128, S], F16)        # roped k, transposed
            # V' tiles: per k-tile j, [128 tokens, 128 ch + ones column]
            vA = consts.tile([128, NJ, 132], BF16)
            for j in range(NJ):
                nc.vector.memset(vA[:, j, 128:129], 1.0)
            attnT = consts.tile([128, QH, S], F16)  # attn out, transposed

            hts_cur = hts0
            for I in range(NI):
                nsl = slice(I * 512, (I + 1) * 512)
                hts = hts_cur

                def rope_into(ps, dst, width=512):
                    """ps: PSUM [128, width] f32 pre-rope (transposed layout).
                    dst: f16 SBUF slice [128, width]. The half-swap crosses
                    partitions, so it runs as a PE matmul against a constant
                    128x128 half-rotation permutation matrix."""
                    cpy = rtmp.tile([128, 512], F16, tag="ropecpy")
                    nc.scalar.copy(cpy[:, :width], ps)
                    sw_ps = popsum.tile([128, 512], F32, tag="opsum")
                    nc.tensor.matmul(
                        sw_ps[:, :width], pswap_sb, cpy[:, :width],
                        start=True, stop=True,
                    )
                    sw = rtmp.tile([128, 512], F16, tag="ropesw")
                    nc.scalar.copy(sw[:, :width], sw_ps[:, :width])
                    tmp2 = rtmp.tile([128, 512], F16, tag="ropecos")
                    nc.vector.tensor_mul(
                        tmp2[:, :width], cpy[:, :width], cos_sb[:, nsl]
                    )
                    nc.vector.tensor_mul(
                        sw[:, :width], sw[:, :width], sin_sb[:, nsl]
                    )
                    nc.vector.tensor_add(dst, tmp2[:, :width], sw[:, :width])

                # ---- projections: k and v first (their weights are
                # small and land early), then the four q heads ----------
                k_ps = pproj.tile([128, 512], F32, tag="proj")
                for d in range(ND):
                    nc.tensor.matmul(
                        k_ps, wk_sb[:, d, :], hts[d // 4][:, d % 4, :],
                        start=(d == 0), stop=(d == ND - 1),
                    )
                rope_into(k_ps, kT[:, nsl])

                v_ps = pproj.tile([128, 512], F32, tag="proj")
                for d in range(ND):
                    nc.tensor.matmul(
                        v_ps, wv_sb[:, d, :], hts[d // 4][:, d % 4, :],
                        start=(d == 0), stop=(d == ND - 1),
                    )
                vt_sb = small.tile([128, 512], BF16, tag="vt")
                nc.vector.tensor_copy(vt_sb, v_ps)
                for jj in range(4):
                    tps = pwork.tile([128, 128], BF16, tag="work")
                    nc.tensor.transpose(
                        tps, vt_sb[:, jj * 128:(jj + 1) * 128], identbf
                    )
                    nc.vector.tensor_copy(vA[:, 4 * I + jj, 0:128], tps)

                for t in range(QH):
                    q_ps = pproj.tile([128, 512], F32, tag="proj")
                    for d in range(ND):
                        nc.tensor.matmul(
                            q_ps,
                            wq_sb[:, d, t * 128:(t + 1) * 128],
                            hts[d // 4][:, d % 4, :],
                            start=(d == 0),
                            stop=(d == ND - 1),
                        )
                    rope_into(q_ps, qT[:, t, nsl])

                # prefetch next pass's hT slice; the spare hpool slots let
                # the first chunks load while this pass still computes
                if I + 1 < NI:
                    hts_cur = []
                    for dq in range(8):
                        t_h = hpool.tile([128, 4, 512], F16, tag="ht",
                                         name=f"ht_{I + 1}_{dq}")
                        hts_cur.append(t_h)
                        ht_dma(hts_cur, I + 1, dq)

                # ---- attention for q-chunk I: flattened (head, j)
                # loop so S matmuls pipeline across head boundaries ------
                njt = 4 * I + 4   # k-tiles in causal range of this chunk
                state = {}

                def start_head(t, I=I):
                    o_ps = [
                        popsum.tile([128, 132], F32, tag="opsum",
                                    name=f"o_ps_{I}_{t}_{il}")
                        for il in range(4)
                    ]
                    return {"o_ps": o_ps, "oscs": [None] * 4}

                def finalize_il(t, il):
                    # softmax divide, emitted as soon as this query tile's
                    # PV accumulation closes: frees the PSUM accumulator
                    # early so the next head's PV can start.
                    o_ps = state[t]["o_ps"]
                    recip = small.tile([128, 1], F32, tag="recip")
                    nc.vector.reciprocal(recip, o_ps[il][:, 128:129])
                    osc = small.tile([128, 128], F16, tag="osc")
                    nc.vector.tensor_scalar_mul(
                        osc, o_ps[il][:, 0:128], recip
                    )
                    state[t]["oscs"][il] = osc

                def drain(t, jprev, s_ps, I=I):
                    m = jprev - 4 * I
                    q_off = 128 * m if m > 0 else 0
                    pt = ptpool.tile([128, 512], BF16, tag="pt")
                    nc.scalar.activation(
                        pt[:, q_off:512], s_ps[:, q_off:512],
                        mybir.ActivationFunctionType.Exp,
                        scale=SCALE,
                    )
                    if m >= 0:
                        # only the boundary 128-col slice is partially
                        # masked; slices below q_off are never read by PV
                        nc.vector.tensor_mul(
                            pt[:, q_off:q_off + 128],
                            pt[:, q_off:q_off + 128],
                            masks[:, 0, 0:128],
                        )
                    o_ps = state[t]["o_ps"]
                    for il in range(4):
                        i_abs = 4 * I + il
                        if jprev <= i_abs:
                            nc.tensor.matmul(
                                o_ps[il][:, 0:129],
                                pt[:, il * 128:(il + 1) * 128],
                                vA[:, jprev, 0:129],
                                start=(jprev == 0),
                                stop=(jprev == i_abs),
                            )
                            if jprev == i_abs:
                                finalize_il(t, il)

                def head_epilogue(t, I=I):
                    for il in range(4):
                        i_abs = 4 * I + il
                        tps = popsum.tile([128, 132], F16, tag="opsum",
                                          name=f"tps_{I}_{t}_{il}")
                        nc.tensor.transpose(
                            tps[:, 0:128], state[t]["oscs"][il], ident16
                        )
                        nc.vector.tensor_copy(
                            attnT[:, t, i_abs * 128:(i_abs + 1) * 128],
                            tps[:, 0:128],
                        )

                prev = None
                for t in range(QH):
                    state[t] = start_head(t)
                    for j in range(njt):
                        mj = j - 4 * I
                        q_off = 128 * mj if mj > 0 else 0
                        s_ps = pwork.tile([128, 512], F32, tag="work")
                        nc.tensor.matmul(
                            s_ps[:, q_off:512],
                            kT[:, j * 128:(j + 1) * 128],
                            qT[:, t, I * 512 + q_off:(I + 1) * 512],
                            start=True, stop=True,
                        )
                        if prev is not None:
                            tp_, jp_, sp_ = prev
                            drain(tp_, jp_, sp_)
                            if jp_ == njt - 1:
                                head_epilogue(tp_)
                        prev = (t, j, s_ps)
                tp_, jp_, sp_ = prev
                drain(tp_, jp_, sp_)
                head_epilogue(tp_)

                # ---- output projection for the 4 token tiles ----------
                for il in range(4):
                    i_abs = 4 * I + il
                    for qtr in range(4):
                        orow = orowp.tile([128, 1024], F16, tag="orow")
                        for mc in range(2):
                            mq = qtr * 1024 + mc * 512
                            op_ps = pwork.tile([128, 512], F32,
                                               tag="work")
                            for oc in range(QH):
                                nc.tensor.matmul(
                                    op_ps,
                                    attnT[:, oc,
                                          i_abs * 128:(i_abs + 1) * 128],
                                    wo_sb[:, oc, mq:mq + 512],
                                    start=(oc == 0),
                                    stop=(oc == QH - 1),
                                )
                            nc.vector.tensor_copy(
                                orow[:, mc * 512:(mc + 1) * 512], op_ps
                            )
                        nc.sync.dma_start(
                            out=out[i_abs * 128:(i_abs + 1) * 128,
                                    qtr * 1024:(qtr + 1) * 1024],
                            in_=orow,
                        )
    nc.finalize()
    return nc


def _pack_inputs(h, position_ids, wq, wk, wv, wo):
    """Host-side shard + transpose + cast. Returns per-core input maps."""
    ht = np.ascontiguousarray(
        h.T.reshape(ND, 128, S).transpose(1, 0, 2)
    ).astype(np.float16)

    # RoPE tables in transposed orientation, halves duplicated / sign-folded.
    inv = 1.0 / (1e6 ** (np.arange(0, HD, 2, dtype=np.float64) / HD))
    fr = position_ids.astype(np.float64)[None, :] * inv[:, None]   # [64, S]
    cosT = np.cos(fr).astype(np.float16)
    sinT = np.sin(fr).astype(np.float16)
    cosd = np.concatenate([cosT, cosT], axis=0)                    # [128, S]
    sind = np.concatenate([-sinT, sinT], axis=0)
    psw = np.zeros((128, 128), dtype=np.float16)
    psw[(np.arange(128) + 64) % 128, np.arange(128)] = 1.0
    import ml_dtypes
    iden16 = np.eye(128, dtype=np.float16)
    idenbf = np.eye(128).astype(ml_dtypes.bfloat16)
    p_i = np.arange(128)[:, None]
    f_i = np.arange(512)[None, :]
    maskd = np.stack(
        [(f_i - p_i - 128 * m >= 0) for m in range(4)], axis=1
    ).astype(ml_dtypes.bfloat16)

    in_maps = []
    for c in range(NCORES):
        wq_c = wq[c * 512:(c + 1) * 512, :]          # [512, HID]
        wk_c = wk[c * 128:(c + 1) * 128, :]
        wv_c = wv[c * 128:(c + 1) * 128, :]
        wo_c = wo[:, c * 512:(c + 1) * 512]          # [HID, 512]
        in_maps.append({
            "ht": ht,
            "wq": np.ascontiguousarray(
                wq_c.T.reshape(ND, 128, 512).transpose(1, 0, 2)
            ).astype(np.float16),
            "wk": np.ascontiguousarray(
                wk_c.T.reshape(ND, 128, 128).transpose(1, 0, 2)
            ).astype(np.float16),
            "wv": np.ascontiguousarray(
                wv_c.T.reshape(ND, 128, 128).transpose(1, 0, 2)
            ).astype(np.float16),
            "wo": np.ascontiguousarray(
                wo_c.T.reshape(QH, 128, HID).transpose(1, 0, 2)
            ).astype(np.float16),
            "cosd": cosd,
            "sind": sind,
            "pswap": psw,
            "identd": iden16,
            "identb": idenbf,
            "maskd": maskd,
        })
    return in_maps


def kernel(h, position_ids, wq, wk, wv, wo):
    global LAST_RESULTS
    from concourse.bass_utils import run_bass_kernel_spmd

    if "nc" not in _CACHE:
        _CACHE["nc"] = _build_program()
    nc = _CACHE["nc"]

    in_maps = _pack_inputs(
        np.asarray(h, dtype=np.float32),
        np.asarray(position_ids),
        np.asarray(wq, dtype=np.float32),
        np.asarray(wk, dtype=np.float32),
        np.asarray(wv, dtype=np.float32),
        np.asarray(wo, dtype=np.float32),
    )

    trace = bool(int(os.environ.get("KERNEL_TRACE", "0")))
    res = run_bass_kernel_spmd(
        nc, in_maps, core_ids=list(range(NCORES)), trace=trace
    )
    LAST_RESULTS = res

    acc = np.zeros((S, HID), dtype=np.float32)
    for r in res.results:
        acc += r["out"].astype(np.float32)
    return acc



# revision 8
# speedup vs baseline: 1.1616x; 1.1616x over previous
"""Mixtral GQA attention block (B=1, S=2048, HID=4096, NH=32, NKV=8, HD=128),
8-way tensor-parallel over heads on trn2: each core owns 4 query heads + 1 KV
head (one GQA group), computes its partial output projection, host sums the
8 partials.

Device layout notes:
  - All matmul operands are staged transposed (contraction dim on partitions).
    Host pre-packs every tensor partition-major so DMAs are identity copies.
  - The four projection GEMMs (Q/K/V/O) run as error-compensated fp8
    DoubleRow matmuls: each operand X is split into X1 = fp8(X) and the
    fp8-quantized residual X2 = fp8(X - X1), stored slot-interleaved per
    128-deep contraction chunk. Per chunk pair, one DoubleRow "pair"
    instruction accumulates X1[2c]Y1[2c] + X1[2c+1]Y1[2c+1] and per chunk one
    "cross" instruction accumulates X1[c]Y2[c] + X2[c]Y1[c]; the dropped
    X2Y2 term is O(4e-4) relative. DoubleRow costs half a PE row per output
    column with a 256-deep contraction, so the scheme runs at 0.75x the fp16
    row count with ~1e-3 GEMM error.
  - Scores are computed transposed (S^T[k,q]) in bf16 so the exp'd tiles
    directly serve as lhsT for the P@V matmul; softmax denominator comes from
    an appended ones-column on V; no max-subtraction (bf16 probabilities
    cannot overflow; scores are O(20)).
  - Causality: only k-tiles j with j*128 <= q_max are computed; the
    diagonal-band blocks use precomputed 0/1 multiplicative masks.
  - RoPE is applied in the transposed orientation; the half-swap crosses
    partitions so it runs as a PE matmul against a constant 128x128
    half-rotation permutation matrix.
"""

import math
import os
import sys

import numpy as np

sys.path.insert(0, "/opt/trn_rl_repo")

import concourse.bass as bass
import concourse.tile as tile
from concourse import bacc
from concourse import mybir

S = 2048
HID = 4096
NH, NKV, HD = 32, 8, 128
NCORES = 8
QH = NH // NCORES      # 4 query heads per core
ND = HID // 128        # 32 contraction chunks
NI = S // 512          # 4 q-chunks of 512
NJ = S // 128          # 16 k-tiles of 128
SCALE = 1.0 / math.sqrt(HD)
# fp8 operand pre-scales: weights/activations are scaled into e4m3's normal
# range on host (w*0.02 and residuals are subnormal otherwise); the inverse
# is folded into the RoPE tables, the V ones-column, and the output copy.
WS = 128.0        # weight scale (wq/wk/wv/wo)
HS = 16.0         # activation scale (h)
AS = 16.0         # attn-output scale into the O projection

F16 = mybir.dt.float16
BF16 = mybir.dt.bfloat16
F32 = mybir.dt.float32
FP8 = mybir.dt.float8e4
DR = mybir.MatmulPerfMode.DoubleRow

_CACHE = {}
LAST_RESULTS = None


def _build_program():
    nc = bacc.Bacc()

    # fp8 operand pairs, slot-interleaved along the contraction-chunk axis.
    # lhsT-side tensors carry slots (X1, X2); rhs-side carry (X2, X1) so the
    # same buffer serves both the pair-AP (stride-2 picks X1 of two chunks)
    # and the cross-AP (stride-1 picks both splits of one chunk).
    ht = nc.declare_dram_parameter("ht", [128, ND, 2, S], FP8, isOutput=False)
    wq = nc.declare_dram_parameter("wq", [128, QH, ND, 2, 128], FP8,
                                   isOutput=False)
    wk = nc.declare_dram_parameter("wk", [128, ND, 2, 128], FP8,
                                   isOutput=False)
    wv = nc.declare_dram_parameter("wv", [128, ND, 2, 128], FP8,
                                   isOutput=False)
    wo = nc.declare_dram_parameter("wo", [128, QH, 2, HID], FP8,
                                   isOutput=False)
    cosd = nc.declare_dram_parameter("cosd", [128, S], F16, isOutput=False)
    pswap = nc.declare_dram_parameter("pswap", [128, 128], F16, isOutput=False)
    identd = nc.declare_dram_parameter("identd", [128, 128], F16,
                                       isOutput=False)
    identb = nc.declare_dram_parameter("identb", [128, 128], BF16,
                                       isOutput=False)
    maskd = nc.declare_dram_parameter("maskd", [128, 4, 512], BF16,
                                      isOutput=False)
    sind = nc.declare_dram_parameter("sind", [128, S], F16, isOutput=False)
    out = nc.declare_dram_parameter("out", [S, HID], F16, isOutput=True)

    with tile.TileContext(nc) as tc:
        with (
            tc.tile_pool(name="consts", bufs=1) as consts,
            tc.tile_pool(name="hpool", bufs=6) as hpool,
            tc.tile_pool(name="qpool", bufs=2) as qpool,
            tc.tile_pool(name="apool", bufs=2) as apool,
            tc.tile_pool(name="ptpool", bufs=5) as ptpool,
            tc.tile_pool(name="rtmp", bufs=3) as rtmp,
            tc.tile_pool(name="small", bufs=8) as small,
            tc.tile_pool(name="orow", bufs=5) as orowp,
            tc.tile_pool(name="pproj", bufs=2, space="PSUM") as pproj,
            tc.tile_pool(name="pwork", bufs=2, space="PSUM") as pwork,
            tc.tile_pool(name="popsum", bufs=4, space="PSUM") as popsum,
        ):
            # First pass activations + first head weights are the startup
            # critical path: enqueue exactly what the first matmuls touch
            # before any bulk weight traffic.
            wq_sb = consts.tile([128, QH, ND, 2, 128], FP8)
            cos_sb = consts.tile([128, S], F16)
            sin_sb = consts.tile([128, S], F16)
            pswap_sb = consts.tile([128, 128], F16)
            wk_sb = consts.tile([128, ND, 2, 128], FP8)
            wv_sb = consts.tile([128, ND, 2, 128], FP8)
            wo_sb = consts.tile([128, QH, 2, HID], FP8)
            ident16 = consts.tile([128, 128], F16)
            identbf = consts.tile([128, 128], BF16)
            masks = consts.tile([128, 4, 512], BF16)

            # 4 ht tiles per 512-token chunk, 8 contraction chunks each
            NHT = 4
            DPT = ND // NHT      # d-chunks per ht tile

            hts0 = []
            for dq in range(NHT):
                t_h = hpool.tile([128, DPT, 2, 512], FP8, tag="ht",
                                 name=f"ht_0_{dq}")
                hts0.append(t_h)

            def ht_dma(tiles, I, dq):
                nc.sync.dma_start(
                    out=tiles[dq],
                    in_=ht[:, dq * DPT:(dq + 1) * DPT, :,
                           I * 512:(I + 1) * 512],
                )

            nc.sync.dma_start(out=wk_sb, in_=wk[:, :, :, :])
            for dq in range(NHT):
                ht_dma(hts0, 0, dq)
            nc.sync.dma_start(out=wv_sb, in_=wv[:, :, :, :])
            nc.sync.dma_start(out=cos_sb, in_=cosd[:, :])
            nc.sync.dma_start(out=sin_sb, in_=sind[:, :])
            nc.sync.dma_start(out=pswap_sb, in_=pswap[:, :])
            nc.sync.dma_start(out=identbf, in_=identb[:, :])
            # 0/1 causal masks for the 4 diagonal-band block offsets
            # (host-built): mask[m][p, f] = 1.0 iff f - p - 128*m >= 0
            nc.sync.dma_start(out=masks, in_=maskd[:, :, :])
            for tt in range(QH):
                nc.sync.dma_start(
                    out=wq_sb[:, tt, :, :, :], in_=wq[:, tt, :, :, :]
                )
            nc.sync.dma_start(out=ident16, in_=identd[:, :])
            for oc in range(QH):
                nc.sync.dma_start(
                    out=wo_sb[:, oc, :, :], in_=wo[:, oc, :, :]
                )

            kT = consts.tile([128, S], F16)        # roped k, transposed
            # V' tiles: per k-tile j, [128 tokens, 128 ch + ones column].
            # V arrives scaled by WS*HS; a ones-column of WS*HS/AS makes the
            # softmax divide emit AS*attn directly.
            vA = consts.tile([128, NJ, 132], BF16)
            for j in range(NJ):
                nc.vector.memset(vA[:, j, 128:129], WS * HS / AS)
            invos = consts.tile([128, 1], F32)     # 1/(AS*WS) output unscale
            nc.vector.memset(invos, 1.0 / (AS * WS))

            def dr_accum(ps, lhs_pair, lhs_cross, rhs_pair, rhs_cross, n):
                """Emit the compensated-fp8 accumulation: n/2 pair instrs
                followed by n cross instrs into PSUM tile ps. lhs/rhs_pair
                and _cross are callables chunk-index -> AP."""
                for c2 in range(n // 2):
                    nc.tensor.matmul(
                        ps, lhs_pair(c2), rhs_pair(c2),
                        start=(c2 == 0), stop=False, perf_mode=DR,
                    )
                for c in range(n):
                    nc.tensor.matmul(
                        ps, lhs_cross(c), rhs_cross(c),
                        start=False, stop=(c == n - 1), perf_mode=DR,
                    )

            hts_cur = hts0
            for I in range(NI):
                nsl = slice(I * 512, (I + 1) * 512)
                hts = hts_cur
                # per-chunk staging: roped q, attn output (f16 + fp8 splits)
                qT = qpool.tile([128, QH, 512], F16, tag="qT",
                                name=f"qT_{I}")
                attnC = apool.tile([128, QH, 512], F16, tag="attnC",
                                   name=f"attnC_{I}")
                attn8 = apool.tile([128, QH, 2, 512], FP8, tag="attn8",
                                   name=f"attn8_{I}")

                def h_pair(c2):
                    lo = (2 * c2) % DPT
                    return hts[(2 * c2) // DPT][:, lo:lo + 2, 1, :]

                def h_cross(c):
                    return hts[c // DPT][:, c % DPT, :, :]

                def rope_into(ps, dst, width=512):
                    """ps: PSUM [128, width] f32 pre-rope (transposed layout).
                    dst: f16 SBUF slice [128, width]. The half-swap crosses
                    partitions, so it runs as a PE matmul against a constant
                    128x128 half-rotation permutation matrix."""
                    cpy = rtmp.tile([128, 512], F16, tag="ropecpy")
                    nc.scalar.copy(cpy[:, :width], ps)
                    sw_ps = popsum.tile([128, 512], F32, tag="opsum")
                    nc.tensor.matmul(
                        sw_ps[:, :width], pswap_sb, cpy[:, :width],
                        start=True, stop=True,
                    )
                    sw = rtmp.tile([128, 512], F16, tag="ropesw")
                    nc.scalar.copy(sw[:, :width], sw_ps[:, :width])
                    tmp2 = rtmp.tile([128, 512], F16, tag="ropecos")
                    nc.vector.tensor_mul(
                        tmp2[:, :width], cpy[:, :width], cos_sb[:, nsl]
                    )
                    nc.vector.tensor_mul(
                        sw[:, :width], sw[:, :width], sin_sb[:, nsl]
                    )
                    nc.vector.tensor_add(dst, tmp2[:, :width], sw[:, :width])

                # ---- projections: k and v first (their weights are
                # small and land early), then the four q heads ----------
                k_ps = pproj.tile([128, 512], F32, tag="proj")
                dr_accum(
                    k_ps,
                    lambda c2: wk_sb[:, 2 * c2:2 * c2 + 2, 0, :],
                    lambda c: wk_sb[:, c, :, :],
                    h_pair, h_cross, ND,
                )
                rope_into(k_ps, kT[:, nsl])

                v_ps = pproj.tile([128, 512], F32, tag="proj")
                dr_accum(
                    v_ps,
                    lambda c2: wv_sb[:, 2 * c2:2 * c2 + 2, 0, :],
                    lambda c: wv_sb[:, c, :, :],
                    h_pair, h_cross, ND,
                )
                vt_sb = small.tile([128, 512], BF16, tag="vt")
                nc.vector.tensor_copy(vt_sb, v_ps)
                for jj in range(4):
                    tps = pwork.tile([128, 128], BF16, tag="work")
                    nc.tensor.transpose(
                        tps, vt_sb[:, jj * 128:(jj + 1) * 128], identbf
                    )
                    nc.vector.tensor_copy(vA[:, 4 * I + jj, 0:128], tps)

                for t in range(QH):
                    q_ps = pproj.tile([128, 512], F32, tag="proj")
                    dr_accum(
                        q_ps,
                        lambda c2, t=t: wq_sb[:, t, 2 * c2:2 * c2 + 2, 0, :],
                        lambda c, t=t: wq_sb[:, t, c, :, :],
                        h_pair, h_cross, ND,
                    )
                    rope_into(q_ps, qT[:, t, :])

                # prefetch next pass's hT slice; the spare hpool slots let
                # the first chunks load while this pass still computes
                if I + 1 < NI:
                    hts_cur = []
                    for dq in range(NHT):
                        t_h = hpool.tile([128, DPT, 2, 512], FP8, tag="ht",
                                         name=f"ht_{I + 1}_{dq}")
                        hts_cur.append(t_h)
                        ht_dma(hts_cur, I + 1, dq)

                # ---- attention for q-chunk I: flattened (head, j)
                # loop so S matmuls pipeline across head boundaries ------
                njt = 4 * I + 4   # k-tiles in causal range of this chunk
                state = {}

                def start_head(t, I=I):
                    o_ps = [
                        popsum.tile([128, 132], F32, tag="opsum",
                                    name=f"o_ps_{I}_{t}_{il}")
                        for il in range(4)
                    ]
                    return {"o_ps": o_ps, "oscs": [None] * 4}

                def finalize_il(t, il):
                    # softmax divide, emitted as soon as this query tile's
                    # PV accumulation closes: frees the PSUM accumulator
                    # early so the next head's PV can start.
                    o_ps = state[t]["o_ps"]
                    recip = small.tile([128, 1], F32, tag="recip")
                    nc.vector.reciprocal(recip, o_ps[il][:, 128:129])
                    osc = small.tile([128, 128], F16, tag="osc")
                    nc.vector.tensor_scalar_mul(
                        osc, o_ps[il][:, 0:128], recip
                    )
                    state[t]["oscs"][il] = osc

                def drain(t, jprev, s_ps, I=I):
                    m = jprev - 4 * I
                    q_off = 128 * m if m > 0 else 0
                    pt = ptpool.tile([128, 512], BF16, tag="pt")
                    nc.scalar.activation(
                        pt[:, q_off:512], s_ps[:, q_off:512],
                        mybir.ActivationFunctionType.Exp,
                        scale=SCALE,
                    )
                    if m >= 0:
                        # only the boundary 128-col slice is partially
                        # masked; slices below q_off are never read by PV
                        nc.vector.tensor_mul(
                            pt[:, q_off:q_off + 128],
                            pt[:, q_off:q_off + 128],
                            masks[:, 0, 0:128],
                        )
                    o_ps = state[t]["o_ps"]
                    for il in range(4):
                        i_abs = 4 * I + il
                        if jprev <= i_abs:
                            nc.tensor.matmul(
                                o_ps[il][:, 0:129],
                                pt[:, il * 128:(il + 1) * 128],
                                vA[:, jprev, 0:129],
                                start=(jprev == 0),
                                stop=(jprev == i_abs),
                            )
                            if jprev == i_abs:
                                finalize_il(t, il)

                def head_epilogue(t, I=I):
                    for il in range(4):
                        tps = popsum.tile([128, 132], F16, tag="opsum",
                                          name=f"tps_{I}_{t}_{il}")
                        nc.tensor.transpose(
                            tps[:, 0:128], state[t]["oscs"][il], ident16
                        )
                        nc.vector.tensor_copy(
                            attnC[:, t, il * 128:(il + 1) * 128],
                            tps[:, 0:128],
                        )
                    # fp8 + residual split feeding the DoubleRow O proj
                    nc.vector.tensor_copy(attn8[:, t, 0, :], attnC[:, t, :])
                    nc.vector.tensor_sub(
                        attn8[:, t, 1, :], attnC[:, t, :], attn8[:, t, 0, :]
                    )

                prev = None
                for t in range(QH):
                    state[t] = start_head(t)
                    for j in range(njt):
                        mj = j - 4 * I
                        q_off = 128 * mj if mj > 0 else 0
                        s_ps = pwork.tile([128, 512], F32, tag="work")
                        nc.tensor.matmul(
                            s_ps[:, q_off:512],
                            kT[:, j * 128:(j + 1) * 128],
                            qT[:, t, q_off:512],
                            start=True, stop=True,
                        )
                        if prev is not None:
                            tp_, jp_, sp_ = prev
                            drain(tp_, jp_, sp_)
                            if jp_ == njt - 1:
                                head_epilogue(tp_)
                        prev = (t, j, s_ps)
                tp_, jp_, sp_ = prev
                drain(tp_, jp_, sp_)
                head_epilogue(tp_)

                # ---- output projection for the 4 token tiles ----------
                for il in range(4):
                    i_abs = 4 * I + il
                    isl = slice(il * 128, (il + 1) * 128)
                    for qtr in range(4):
                        orow = orowp.tile([128, 1024], F16, tag="orow")
                        for mc in range(2):
                            mq = qtr * 1024 + mc * 512
                            op_ps = pwork.tile([128, 512], F32,
                                               tag="work")
                            dr_accum(
                                op_ps,
                                lambda p2: attn8[:, 2 * p2:2 * p2 + 2, 0,
                                                 isl],
                                lambda oc: attn8[:, oc, :, isl],
                                lambda p2, mq=mq: wo_sb[:, 2 * p2:2 * p2 + 2,
                                                        1, mq:mq + 512],
                                lambda oc, mq=mq: wo_sb[:, oc, :,
                                                        mq:mq + 512],
                                QH,
                            )
                            nc.vector.tensor_scalar_mul(
                                orow[:, mc * 512:(mc + 1) * 512], op_ps,
                                invos,
                            )
                        nc.sync.dma_start(
                            out=out[i_abs * 128:(i_abs + 1) * 128,
                                    qtr * 1024:(qtr + 1) * 1024],
                            in_=orow,
                        )
    nc.finalize()
    return nc


def _pack_inputs(h, position_ids, wq, wk, wv, wo):
    """Host-side shard + transpose + fp8/residual split. Returns per-core
    input maps."""
    import ml_dtypes

    E4 = ml_dtypes.float8_e4m3

    def split8(x):
        x1 = x.astype(E4)
        x2 = (x - x1.astype(np.float32)).astype(E4)
        return x1, x2

    def pack_lhs(x1, x2):
        # [..., nc, 128, F] pair -> [128, ..., nc, 2, F] slots (X1, X2)
        st = np.stack([x1, x2], axis=-2)
        return np.ascontiguousarray(np.moveaxis(st, -3, 0))

    def pack_rhs(x1, x2):
        # rhs-side slot order (X2, X1)
        st = np.stack([x2, x1], axis=-2)
        return np.ascontiguousarray(np.moveaxis(st, -3, 0))

    # activations: [S, HID] -> chunks [ND, 128, S], slots (h2, h1)
    hT = np.ascontiguousarray(h.T).reshape(ND, 128, S) * HS
    h1, h2 = split8(hT)
    ht8 = pack_rhs(h1, h2)                               # [128, ND, 2, S]

    # RoPE tables in transposed orientation, halves duplicated / sign-folded;
    # 1/(WS*HS) folded in to undo the fp8 operand pre-scales on q/k.
    inv = 1.0 / (1e6 ** (np.arange(0, HD, 2, dtype=np.float64) / HD))
    fr = position_ids.astype(np.float64)[None, :] * inv[:, None]   # [64, S]
    cosT = (np.cos(fr) / (WS * HS)).astype(np.float16)
    sinT = (np.sin(fr) / (WS * HS)).astype(np.float16)
    cosd = np.concatenate([cosT, cosT], axis=0)                    # [128, S]
    sind = np.concatenate([-sinT, sinT], axis=0)
    psw = np.zeros((128, 128), dtype=np.float16)
    psw[(np.arange(128) + 64) % 128, np.arange(128)] = 1.0
    import ml_dtypes as mld
    iden16 = np.eye(128, dtype=np.float16)
    idenbf = np.eye(128).astype(mld.bfloat16)
    p_i = np.arange(128)[:, None]
    f_i = np.arange(512)[None, :]
    maskd = np.stack(
        [(f_i - p_i - 128 * m >= 0) for m in range(4)], axis=1
    ).astype(mld.bfloat16)

    in_maps = []
    for c in range(NCORES):
        wq_c = wq[c * 512:(c + 1) * 512, :]          # [512, HID]
        wk_c = wk[c * 128:(c + 1) * 128, :]
        wv_c = wv[c * 128:(c + 1) * 128, :]
        wo_c = wo[:, c * 512:(c + 1) * 512]          # [HID, 512]

        # wq lhsT head-major: [QH, ND, 128, 128]
        wq_t = np.ascontiguousarray(
            wq_c.T.reshape(ND, 128, QH, 128).transpose(2, 0, 1, 3)
        )
        wq8 = pack_lhs(*split8(wq_t * WS))           # [128, QH, ND, 2, 128]
        wk8 = pack_lhs(*split8(wk_c.T.reshape(ND, 128, 128) * WS))
        wv8 = pack_lhs(*split8(wv_c.T.reshape(ND, 128, 128) * WS))
        # wo rhs: per head [QH, 128 hd, HID], slots (wo2, wo1)
        wo_t = np.ascontiguousarray(wo_c.T.reshape(QH, 128, HID))
        wo8 = pack_rhs(*split8(wo_t * WS))           # [128, QH, 2, HID]

        in_maps.append({
            "ht": ht8,
            "wq": wq8,
            "wk": wk8,
            "wv": wv8,
            "wo": wo8,
            "cosd": cosd,
            "sind": sind,
            "pswap": psw,
            "identd": iden16,
            "identb": idenbf,
            "maskd": maskd,
        })
    return in_maps


def kernel(h, position_ids, wq, wk, wv, wo):
    global LAST_RESULTS
    from concourse.bass_utils import run_bass_kernel_spmd

    if "nc" not in _CACHE:
        _CACHE["nc"] = _build_program()
    nc = _CACHE["nc"]

    in_maps = _pack_inputs(
        np.asarray(h, dtype=np.float32),
        np.asarray(position_ids),
        np.asarray(wq, dtype=np.float32),
        np.asarray(wk, dtype=np.float32),
        np.asarray(wv, dtype=np.float32),
        np.asarray(wo, dtype=np.float32),
    )

    trace = bool(int(os.environ.get("KERNEL_TRACE", "0")))
    res = run_bass_kernel_spmd(
        nc, in_maps, core_ids=list(range(NCORES)), trace=trace
    )
    LAST_RESULTS = res

    acc = np.zeros((S, HID), dtype=np.float32)
    for r in res.results:
        acc += r["out"].astype(np.float32)
    return acc


# revision 78
# speedup vs baseline: 1.2974x; 1.1170x over previous
"""Mixtral GQA attention block (B=1, S=2048, HID=4096, NH=32, NKV=8, HD=128),
8-way tensor-parallel over heads on trn2: each core owns 4 query heads + 1 KV
head (one GQA group), computes its partial output projection, host sums the
8 partials.

Device layout notes:
  - All matmul operands are staged transposed (contraction dim on partitions).
    Host pre-packs every tensor partition-major so DMAs are identity copies.
  - The four projection GEMMs (Q/K/V/O) run as error-compensated fp8
    DoubleRow matmuls: each operand X is split into X1 = fp8(X) and the
    fp8-quantized residual X2 = fp8(X - X1), stored slot-interleaved per
    128-deep contraction chunk. Per chunk pair, one DoubleRow "pair"
    instruction accumulates X1[2c]Y1[2c] + X1[2c+1]Y1[2c+1] and per chunk one
    "cross" instruction accumulates X1[c]Y2[c] + X2[c]Y1[c]; the dropped
    X2Y2 term is O(4e-4) relative. DoubleRow costs half a PE row per output
    column with a 256-deep contraction, so the scheme runs at 0.75x the fp16
    row count with ~1e-3 GEMM error. Operands are pre-scaled into e4m3's
    normal range (w*0.02 is subnormal otherwise); the inverse scales fold
    into the RoPE tables, the denominator column, and the output unscale.
  - Scores are computed transposed (S^T[k,q]) in bf16 so the exp'd tiles
    directly serve as lhsT for the P@V matmul; no max-subtraction (bf16
    probabilities cannot overflow; scores are O(20)). All four query-tile
    PV accumulators of a head share one PSUM bank (4x128 f32 = 2048B) and
    all 16 softmax denominators of a chunk share another, fed by per-tile
    1-column matmuls; sub-accumulators beyond the first rely on the
    bank-wide pending-zero armed by the first start_tensor_calc. The freed
    banks deepen the score pipeline (drains lag the scores by 5 tiles),
    which hides the exp chain latency; the post-attention output
    projections cycle their accumulators across the then-idle score and
    PV banks.
  - Causality: only k-tiles j with j*128 <= q_max are computed; the
    diagonal-band blocks use a precomputed 0/1 multiplicative mask.
  - RoPE is applied in the transposed orientation; the half-swap crosses
    partitions so it runs as a pair of SBUF-to-SBUF partition-shift DMAs.
  - The emission is software-pipelined: chunk I+1's projection instructions
    are interleaved between chunk I's attention drains so the PE never
    starves while the Activation engine works through the exp stream (the
    attention phase alone is Act-bound). The final chunk's attention instead
    hides chunk 2's output projection, which borrows the then-idle
    projection PSUM slots.
"""

import math
import os
import sys

import numpy as np

sys.path.insert(0, "/opt/trn_rl_repo")

import concourse.bass as bass
import concourse.tile as tile
from concourse import bacc
from concourse import mybir

S = 2048
HID = 4096
NH, NKV, HD = 32, 8, 128
NCORES = 8
QH = NH // NCORES      # 4 query heads per core
ND = HID // 128        # 32 contraction chunks
NI = S // 512          # 4 q-chunks of 512
NJ = S // 128          # 16 k-tiles of 128
SCALE = 1.0 / math.sqrt(HD)
# fp8 operand pre-scales: weights/activations are scaled into e4m3's normal
# range on host (w*0.02 and residuals are subnormal otherwise); the inverse
# is folded into the RoPE tables, the denominator column, and the output
# unscale.
WS = 128.0        # weight scale (wq/wk/wv/wo)
HS = 16.0         # activation scale (h)
AS = 16.0         # attn-output scale into the O projection

F16 = mybir.dt.float16
BF16 = mybir.dt.bfloat16
F32 = mybir.dt.float32
FP8 = mybir.dt.float8e4
DR = mybir.MatmulPerfMode.DoubleRow

_CACHE = {}
LAST_RESULTS = None
# dev knob: 1 = interleave next chunk's projections into the attention
# drains; 0 = emit them as a block before the attention phase
INTERLEAVE = int(os.environ.get("KERNEL_INTERLEAVE", "1"))
# dev knob: 1 = mask-mul + attn8-split on GpSimd, 0 = DVE (v2)
GPS = int(os.environ.get("KERNEL_GPS", "0"))
UNSCALE_ACT = int(os.environ.get("KERNEL_UNSCALE_ACT", "0"))


def _build_program():
    nc = bacc.Bacc()

    # fp8 operand pairs, slot-interleaved along the contraction-chunk axis.
    # lhsT-side tensors carry slots (X1, X2); rhs-side carry (X2, X1) so the
    # same buffer serves both the pair-AP (stride-2 picks X1 of two chunks)
    # and the cross-AP (stride-1 picks both splits of one chunk).
    ht = nc.declare_dram_parameter("ht", [128, ND, 2, S], FP8, isOutput=False)
    wq = nc.declare_dram_parameter("wq", [128, QH, ND, 2, 128], FP8,
                                   isOutput=False)
    wk = nc.declare_dram_parameter("wk", [128, ND, 2, 128], FP8,
                                   isOutput=False)
    wv = nc.declare_dram_parameter("wv", [128, ND, 2, 128], FP8,
                                   isOutput=False)
    wo = nc.declare_dram_parameter("wo", [128, QH, 2, HID], FP8,
                                   isOutput=False)
    cosd = nc.declare_dram_parameter("cosd", [128, S], F16, isOutput=False)
    identd = nc.declare_dram_parameter("identd", [128, 128], F16,
                                       isOutput=False)
    identb = nc.declare_dram_parameter("identb", [128, 128], BF16,
                                       isOutput=False)
    maskd = nc.declare_dram_parameter("maskd", [128, 128], BF16,
                                      isOutput=False)
    sind = nc.declare_dram_parameter("sind", [128, S], F16, isOutput=False)
    out = nc.declare_dram_parameter("out", [S, HID], F16, isOutput=True)

    with tile.TileContext(nc) as tc:
        with (
            tc.tile_pool(name="consts", bufs=1) as consts,
            tc.tile_pool(name="hpool", bufs=6) as hpool,
            tc.tile_pool(name="qpool", bufs=2) as qpool,
            tc.tile_pool(name="apool", bufs=2) as apool,
            tc.tile_pool(name="ptpool", bufs=PT) as ptpool,
            tc.tile_pool(name="rtmp", bufs=3) as rtmp,
            tc.tile_pool(name="small", bufs=8) as small,
            tc.tile_pool(name="orow", bufs=5) as orowp,
            tc.tile_pool(name="pproj", bufs=2, space="PSUM") as pproj,
            tc.tile_pool(name="pwork", bufs=3, space="PSUM") as pwork,
            tc.tile_pool(name="pops", bufs=2, space="PSUM") as pops,
            tc.tile_pool(name="denp", bufs=1, space="PSUM") as denp,
        ):
            # First pass activations + first head weights are the startup
            # critical path: enqueue exactly what the first matmuls touch
            # before any bulk weight traffic.
            wq_sb = consts.tile([128, QH, ND, 2, 128], FP8)
            cos_sb = consts.tile([128, S], F16)
            sin_sb = consts.tile([128, S], F16)
            wk_sb = consts.tile([128, ND, 2, 128], FP8)
            wv_sb = consts.tile([128, ND, 2, 128], FP8)
            wo_sb = consts.tile([128, QH, 2, HID], FP8)
            masks = consts.tile([128, 128], BF16)
            ident16 = consts.tile([128, 128], F16)
            identbf = consts.tile([128, 128], BF16)

            # 4 ht tiles per 512-token chunk, 8 contraction chunks each
            NHT = 4
            DPT = ND // NHT      # d-chunks per ht tile

            hts_all = {}

            def ht_dma(I, dqs=None):
                tiles = hts_all.setdefault(I, {})
                for dq in (range(NHT) if dqs is None else dqs):
                    t_h = hpool.tile([128, DPT, 2, 512], FP8, tag="ht",
                                     name=f"ht_{I}_{dq}")
                    tiles[dq] = t_h
                    nc.sync.dma_start(
                        out=t_h,
                        in_=ht[:, dq * DPT:(dq + 1) * DPT, :,
                               I * 512:(I + 1) * 512],
                    )

            # startup DMA order matches the tile-major chunk-0 emission:
            # each ht tile unlocks one more contraction quarter for every
            # projection, and each wq head arrives just before its first
            # quarter is emitted.
            nc.sync.dma_start(out=wk_sb, in_=wk[:, :, :, :])
            ht_dma(0, [0])
            nc.sync.dma_start(out=wv_sb, in_=wv[:, :, :, :])
            nc.sync.dma_start(out=wq_sb[:, 0, :, :, :], in_=wq[:, 0, :, :, :])
            ht_dma(0, [1])
            nc.sync.dma_start(out=wq_sb[:, 1, :, :, :], in_=wq[:, 1, :, :, :])
            ht_dma(0, [2])
            nc.sync.dma_start(out=wq_sb[:, 2, :, :, :], in_=wq[:, 2, :, :, :])
            ht_dma(0, [3])
            nc.sync.dma_start(out=wq_sb[:, 3, :, :, :], in_=wq[:, 3, :, :, :])
            nc.sync.dma_start(out=cos_sb, in_=cosd[:, :])
            nc.sync.dma_start(out=sin_sb, in_=sind[:, :])
            nc.sync.dma_start(out=identbf, in_=identb[:, :])
            nc.sync.dma_start(out=ident16, in_=identd[:, :])
            nc.sync.dma_start(out=masks, in_=maskd[:, :])


            kT = consts.tile([128, S], F16)        # roped k, transposed
            # V' tiles: per k-tile j, [128 tokens, 128 channels]. V arrives
            # scaled by WS*HS; the denominator matmuls use a WS*HS/AS column
            # so the softmax divide emits AS*attn directly.
            vA = consts.tile([128, NJ, 128], BF16)
            onecol = consts.tile([128, 1], BF16)
            nc.vector.memset(onecol, WS * HS / AS)
            invos = consts.tile([128, 1], F32)     # 1/(AS*WS) output unscale
            nc.vector.memset(invos, 1.0 / (AS * WS))

            qTs = {}

            def dr_accum(ps, lhs_pair, lhs_cross, rhs_pair, rhs_cross, n,
                         emit=None):
                """Emit (or queue as thunks) the compensated-fp8
                accumulation: n/2 pair instrs followed by n cross instrs
                into PSUM tile ps."""
                def mk_pair(c2):
                    return lambda: nc.tensor.matmul(
                        ps, lhs_pair(c2), rhs_pair(c2),
                        start=(c2 == 0), stop=False, perf_mode=DR,
                    )

                def mk_cross(c):
                    return lambda: nc.tensor.matmul(
                        ps, lhs_cross(c), rhs_cross(c),
                        start=False, stop=(c == n - 1), perf_mode=DR,
                    )

                ths = [mk_pair(c2) for c2 in range(n // 2)]
                ths += [mk_cross(c) for c in range(n)]
                if emit is None:
                    for th in ths:
                        th()
                else:
                    emit.extend(ths)

            def rope_into(ps, dst, nsl, name, width=512):
                """ps: PSUM [128, width] f32 pre-rope (transposed layout).
                dst: f16 SBUF slice [128, width]. The half-swap crosses
                partitions, so it runs as a pair of SBUF-to-SBUF
                partition-shift DMAs issued from the idle Act sequencer;
                the sign fold lives in the sin table. Latency is off the
                critical path (results are consumed a phase later).
                Returns thunks."""
                cpy = rtmp.tile([128, 512], F16, tag="ropecpy",
                                name=f"rc_{name}")
                sw = rtmp.tile([128, 512], F16, tag="ropesw",
                               name=f"rs_{name}")
                tmp2 = rtmp.tile([128, 512], F16, tag="ropecos",
                                 name=f"rm_{name}")

                def t1():
                    nc.vector.tensor_copy(cpy[:, :width], ps)

                def t2():
                    nc.sync.dma_start(out=sw[0:64, :width],
                                        in_=cpy[64:128, :width])
                    nc.sync.dma_start(out=sw[64:128, :width],
                                        in_=cpy[0:64, :width])

                def t3():
                    nc.vector.tensor_mul(
                        sw[:, :width], sw[:, :width], sin_sb[:, nsl]
                    )
                    nc.vector.tensor_mul(
                        tmp2[:, :width], cpy[:, :width], cos_sb[:, nsl]
                    )
                    nc.vector.tensor_add(dst, tmp2[:, :width], sw[:, :width])

                return [t1, t2, t3]

            def proj_stream(I):
                """Thunk list computing chunk I's projections + ropes, and
                prefetching chunk I+1's activations at the tail."""
                ths = []
                nsl = slice(I * 512, (I + 1) * 512)
                hts = hts_all[I]
                qT = qpool.tile([128, QH, 512], F16, tag="qT",
                                name=f"qT_{I}")
                qTs[I] = qT

                def h_pair(c2):
                    lo = (2 * c2) % DPT
                    return hts[(2 * c2) // DPT][:, lo:lo + 2, 1, :]

                def h_cross(c):
                    return hts[c // DPT][:, c % DPT, :, :]

                k_ps = pproj.tile([128, 512], F32, tag="proj",
                                  name=f"k_ps_{I}")
                dr_accum(
                    k_ps,
                    lambda c2: wk_sb[:, 2 * c2:2 * c2 + 2, 0, :],
                    lambda c: wk_sb[:, c, :, :],
                    h_pair, h_cross, ND, emit=ths,
                )
                ths += rope_into(k_ps, kT[:, nsl], nsl, f'k{I}')

                v_ps = pproj.tile([128, 512], F32, tag="proj",
                                  name=f"v_ps_{I}")
                dr_accum(
                    v_ps,
                    lambda c2: wv_sb[:, 2 * c2:2 * c2 + 2, 0, :],
                    lambda c: wv_sb[:, c, :, :],
                    h_pair, h_cross, ND, emit=ths,
                )
                vt_sb = small.tile([128, 512], BF16, tag="vt",
                                   name=f"vt_{I}")

                def vt_copy():
                    nc.vector.tensor_copy(vt_sb, v_ps)

                def mk_vtrans_fwd(jj):
                    def f():
                        tps = pwork.tile([128, 128], BF16, tag="work",
                                         name=f"vtps_{I}_{jj}")
                        nc.tensor.transpose(
                            tps, vt_sb[:, jj * 128:(jj + 1) * 128], identbf
                        )
                        nc.vector.tensor_copy(vA[:, 4 * I + jj, 0:128], tps)
                    return f

                ths.append(vt_copy)

                for t in range(QH):
                    q_ps = pproj.tile([128, 512], F32, tag="proj",
                                      name=f"q_ps_{I}_{t}")
                    dr_accum(
                        q_ps,
                        lambda c2, t=t: wq_sb[:, t, 2 * c2:2 * c2 + 2, 0, :],
                        lambda c, t=t: wq_sb[:, t, c, :, :],
                        h_pair, h_cross, ND, emit=ths,
                    )
                    ths += rope_into(q_ps, qT[:, t, :], nsl, f'q{I}{t}')
                    if t == 0:
                        ths += [mk_vtrans_fwd(jj) for jj in range(4)]

                if I + 1 < NI:
                    ths.append(lambda: ht_dma(I + 1))
                return ths

            def oproj_chunk(I, attn8, pools, emit=None,
                            act_ok=True, pump_cb=None):
                """Output projection for chunk I's 4 token tiles. op
                accumulators alternate across the given (pool, tag) list
                for a deeper effective PSUM cycle."""
                ths = []
                gidx = [0]
                for il in range(4):
                    i_abs = 4 * I + il
                    isl = slice(il * 128, (il + 1) * 128)
                    for qtr in range(4):
                        orow = orowp.tile([128, 1024], F16, tag="orow",
                                          name=f"orow_{I}_{il}_{qtr}")
                        for mc in range(2):
                            mq = qtr * 1024 + mc * 512
                            pool, ptag = pools[gidx[0] % len(pools)]
                            gidx[0] += 1
                            op_ps = pool.tile(
                                [128, 512], F32, tag=ptag,
                                name=f"op_ps_{I}_{il}_{qtr}_{mc}")
                            dr_accum(
                                op_ps,
                                lambda p2, isl=isl: attn8[
                                    :, 2 * p2:2 * p2 + 2, 0, isl],
                                lambda oc, isl=isl: attn8[:, oc, :, isl],
                                lambda p2, mq=mq: wo_sb[:, 2 * p2:2 * p2 + 2,
                                                        1, mq:mq + 512],
                                lambda oc, mq=mq: wo_sb[:, oc, :,
                                                        mq:mq + 512],
                                QH, emit=ths,
                            )

                            def unscale(op_ps=op_ps, orow=orow, mc=mc):
                                # split across DVE and Act; Act is idle
                                # during O-proj phases (act_ok=False for the
                                # chunk hidden inside the exp-heavy last
                                # attention phase)
                                if mc == 1 and act_ok and UNSCALE_ACT:
                                    nc.scalar.activation(
                                        orow[:, 512:1024], op_ps,
                                        mybir.ActivationFunctionType.Copy,
                                        scale=1.0 / (AS * WS),
                                    )
                                else:
                                    nc.vector.tensor_scalar_mul(
                                        orow[:, mc * 512:(mc + 1) * 512],
                                        op_ps, invos,
                                    )

                            ths.append(unscale)

                        def send(orow=orow, i_abs=i_abs, qtr=qtr):
                            nc.sync.dma_start(
                                out=out[i_abs * 128:(i_abs + 1) * 128,
                                        qtr * 1024:(qtr + 1) * 1024],
                                in_=orow,
                            )

                        ths.append(send)
                if emit is None:
                    for i, th in enumerate(ths):
                        th()
                        if pump_cb is not None and i % 8 == 7:
                            pump_cb()
                else:
                    emit.extend(ths)

            # ---- startup: chunk 0 projections emitted directly, in
            # ht-TILE-major order so each projection consumes activation
            # tiles as the serial startup DMAs land (all six accumulators
            # live at once across the then-idle PSUM pools) --------------
            def proj_stream0():
                I = 0
                nsl = slice(0, 512)
                hts = hts_all[0]
                qT = qpool.tile([128, QH, 512], F16, tag="qT", name="qT_0")
                qTs[0] = qT

                k_ps = pproj.tile([128, 512], F32, tag="proj",
                                  name="k_ps_0")
                v_ps = pproj.tile([128, 512], F32, tag="proj",
                                  name="v_ps_0")
                q_pss = [
                    pwork.tile([128, 512], F32, tag="work",
                               name=f"q_ps_0_{t}") for t in range(3)
                ] + [pops.tile([128, 512], F32, tag="ops", name="q_ps_0_3")]

                def seg(ps, lhs_pair, lhs_cross, dq, first, last):
                    for c2 in range(4 * dq, 4 * dq + 4):
                        lo = (2 * c2) % DPT
                        nc.tensor.matmul(
                            ps, lhs_pair(c2),
                            hts[(2 * c2) // DPT][:, lo:lo + 2, 1, :],
                            start=(first and c2 == 4 * dq), stop=False,
                            perf_mode=DR,
                        )
                    for c in range(DPT * dq, DPT * dq + DPT):
                        nc.tensor.matmul(
                            ps, lhs_cross(c),
                            hts[c // DPT][:, c % DPT, :, :],
                            start=False,
                            stop=(last and c == DPT * dq + DPT - 1),
                            perf_mode=DR,
                        )

                for dq in range(NHT):
                    first, last = (dq == 0), (dq == NHT - 1)
                    seg(k_ps,
                        lambda c2: wk_sb[:, 2 * c2:2 * c2 + 2, 0, :],
                        lambda c: wk_sb[:, c, :, :], dq, first, last)
                    if last:
                        for th in rope_into(k_ps, kT[:, nsl], nsl, 'k0'):
                            th()
                    seg(v_ps,
                        lambda c2: wv_sb[:, 2 * c2:2 * c2 + 2, 0, :],
                        lambda c: wv_sb[:, c, :, :], dq, first, last)
                    if last:
                        vt_sb = small.tile([128, 512], BF16, tag="vt",
                                           name="vt_0")
                        nc.vector.tensor_copy(vt_sb, v_ps)
                    for t in range(QH):
                        seg(q_pss[t],
                            lambda c2, t=t: wq_sb[:, t, 2 * c2:2 * c2 + 2,
                                                  0, :],
                            lambda c, t=t: wq_sb[:, t, c, :, :],
                            dq, first, last)
                        if last:
                            for th in rope_into(q_pss[t], qT[:, t, :], nsl,
                                                f'q0{t}'):
                                th()
                for jj in range(4):
                    tps = pwork.tile([128, 128], BF16, tag="work",
                                     name=f"vtps_0_{jj}")
                    nc.tensor.transpose(
                        tps, vt_sb[:, jj * 128:(jj + 1) * 128], identbf
                    )
                    nc.vector.tensor_copy(vA[:, jj, 0:128], tps)
                ht_dma(1)

            proj_stream0()

            attn8s = {}
            for I in range(NI):
                qT = qTs[I]
                attnC = apool.tile([128, QH, 512], F16, tag="attnC",
                                   name=f"attnC_{I}")
                attn8 = apool.tile([128, QH, 2, 512], FP8, tag="attn8",
                                   name=f"attn8_{I}")
                attn8s[I] = attn8

                # interleave stream: next chunk's projections, or (for the
                # last chunk) the previous chunk's output projection.
                if I + 1 < NI:
                    stream = proj_stream(I + 1)
                else:
                    stream = []
                    oproj_chunk(NI - 2, attn8s[NI - 2], [(pproj, 'proj')],
                                emit=stream,
                                act_ok=False)
                if not INTERLEAVE:
                    for th in stream:
                        th()
                    stream = []

                # ---- attention for q-chunk I: flattened (head, j)
                # loop so S matmuls pipeline across head boundaries ------
                njt = 4 * I + 4   # k-tiles in causal range of this chunk
                ndrains = QH * njt
                # pump points span the attention drains plus (for chunks
                # with a trailing output projection) the O-proj groups, so
                # a short attention phase doesn't dump the stream onto
                # not-yet-loaded activations.
                npoints = ndrains + (32 * 10 // 8 if I < NI - 2 else 0)
                emitted = [0]
                drained = [0]

                def pump():
                    drained[0] += 1
                    want = (len(stream) * drained[0]) // npoints
                    while emitted[0] < want and emitted[0] < len(stream):
                        stream[emitted[0]]()
                        emitted[0] += 1

                state = {}
                # all 16 softmax denominators of the chunk accumulate into
                # one PSUM bank; all 4 query-tile PV accumulators of a head
                # share another (2048B exactly). Sub-accumulators beyond the
                # first rely on the bank-wide pending-zero armed by the
                # first start_tensor_calc, so the group checker is skipped.
                den_all = denp.tile([128, QH, 4], F32, tag="den",
                                    name=f"den_{I}")

                def start_head(t, I=I):
                    o_ps = pops.tile([128, 4, 128], F32, tag="ops",
                                     name=f"o_ps_{I}_{t}")
                    return {"o": o_ps, "oscs": [None] * 4}

                def finalize_il(t, il, I=I, den_all=den_all):
                    # softmax divide, emitted as soon as this query tile's
                    # PV accumulation closes.
                    recip = small.tile([128, 1], F32, tag="recip")
                    nc.vector.reciprocal(recip, den_all[:, t, il:il + 1])
                    osc = small.tile([128, 128], F16, tag="osc")
                    nc.vector.tensor_scalar_mul(
                        osc, state[t]["o"][:, il, :], recip
                    )
                    state[t]["oscs"][il] = osc

                def drain(t, jprev, s_ps, I=I, den_all=den_all):
                    m = jprev - 4 * I
                    q_off = 128 * m if m > 0 else 0
                    pt = ptpool.tile([128, 512], BF16, tag="pt")
                    nc.scalar.activation(
                        pt[:, q_off:512], s_ps[:, q_off:512],
                        mybir.ActivationFunctionType.Exp,
                        scale=SCALE,
                    )
                    if m >= 0:
                        # only the boundary 128-col slice is partially
                        # masked; slices below q_off are never read by PV
                        nc.vector.tensor_mul(
                            pt[:, q_off:q_off + 128],
                            pt[:, q_off:q_off + 128],
                            masks[:, 0:128],
                        )
                    o_ps = state[t]["o"]
                    njt = 4 * I + 4
                    for il in range(4):
                        i_abs = 4 * I + il
                        if jprev <= i_abs:
                            ptsl = pt[:, il * 128:(il + 1) * 128]
                            nc.tensor.matmul(
                                o_ps[:, il, :],
                                ptsl, vA[:, jprev, 0:128],
                                start=(jprev == 0 and il == 0),
                                stop=(jprev == i_abs and il == 3),
                                skip_group_check=True,
                            )
                            nc.tensor.matmul(
                                den_all[:, t, il:il + 1],
                                ptsl, onecol,
                                start=(t == 0 and il == 0 and jprev == 0),
                                stop=(t == QH - 1 and il == 3
                                      and jprev == njt - 1),
                                skip_group_check=True,
                            )
                            if jprev == i_abs:
                                finalize_il(t, il)

                def head_epilogue(t, I=I, attnC=attnC, attn8=attn8):
                    for il in range(4):
                        tps = pwork.tile([128, 132], F16, tag="work",
                                         name=f"tps_{I}_{t}_{il}")
                        nc.tensor.transpose(
                            tps[:, 0:128], state[t]["oscs"][il], ident16
                        )
                        nc.vector.tensor_copy(
                            attnC[:, t, il * 128:(il + 1) * 128],
                            tps[:, 0:128],
                        )
                    # fp8 + residual split feeding the DoubleRow O proj
                    eng = nc.gpsimd if GPS else nc.vector
                    eng.tensor_copy(attn8[:, t, 0, :], attnC[:, t, :])
                    eng.tensor_sub(
                        attn8[:, t, 1, :], attnC[:, t, :], attn8[:, t, 0, :]
                    )

                prev = None
                for t in range(QH):
                    state[t] = start_head(t)
                    for j in range(njt):
                        mj = j - 4 * I
                        q_off = 128 * mj if mj > 0 else 0
                        s_ps = pwork.tile([128, 512], F32, tag="work")
                        nc.tensor.matmul(
                            s_ps[:, q_off:512],
                            kT[:, j * 128:(j + 1) * 128],
                            qT[:, t, q_off:512],
                            start=True, stop=True,
                        )
                        if prev is not None:
                            tp_, jp_, sp_ = prev
                            pump()
                            drain(tp_, jp_, sp_)
                            if jp_ == njt - 1:
                                head_epilogue(tp_)
                        prev = (t, j, s_ps)
                tp_, jp_, sp_ = prev
                pump()
                drain(tp_, jp_, sp_)
                head_epilogue(tp_)

                # ---- output projection (chunks other than NI-2, which is
                # interleaved into the last attention phase); keeps pumping
                # the projection stream between groups -------------------
                if I < NI - 2:
                    oproj_chunk(I, attn8,
                                [(pwork, 'work'), (pops, 'ops')],
                                pump_cb=pump)
                while emitted[0] < len(stream):
                    stream[emitted[0]]()
                    emitted[0] += 1
            oproj_chunk(NI - 1, attn8s[NI - 1],
                        [(pwork, 'work'), (pops, 'ops')])
    nc.finalize()
    return nc


def _pack_inputs(h, position_ids, wq, wk, wv, wo):
    """Host-side shard + transpose + fp8/residual split. Returns per-core
    input maps."""
    import ml_dtypes

    E4 = ml_dtypes.float8_e4m3

    def split8(x):
        x1 = x.astype(E4)
        x2 = (x - x1.astype(np.float32)).astype(E4)
        return x1, x2

    def pack_lhs(x1, x2):
        # [..., nc, 128, F] pair -> [128, ..., nc, 2, F] slots (X1, X2)
        st = np.stack([x1, x2], axis=-2)
        return np.ascontiguousarray(np.moveaxis(st, -3, 0))

    def pack_rhs(x1, x2):
        # rhs-side slot order (X2, X1)
        st = np.stack([x2, x1], axis=-2)
        return np.ascontiguousarray(np.moveaxis(st, -3, 0))

    # activations: [S, HID] -> chunks [ND, 128, S], slots (h2, h1)
    hT = np.ascontiguousarray(h.T).reshape(ND, 128, S) * HS
    h1, h2 = split8(hT)
    ht8 = pack_rhs(h1, h2)                               # [128, ND, 2, S]

    # RoPE tables in transposed orientation, halves duplicated / sign-folded;
    # 1/(WS*HS) folded in to undo the fp8 operand pre-scales on q/k.
    inv = 1.0 / (1e6 ** (np.arange(0, HD, 2, dtype=np.float64) / HD))
    fr = position_ids.astype(np.float64)[None, :] * inv[:, None]   # [64, S]
    cosT = (np.cos(fr) / (WS * HS)).astype(np.float16)
    sinT = (np.sin(fr) / (WS * HS)).astype(np.float16)
    cosd = np.concatenate([cosT, cosT], axis=0)                    # [128, S]
    sind = np.concatenate([-sinT, sinT], axis=0)
    import ml_dtypes as mld
    iden16 = np.eye(128, dtype=np.float16)
    idenbf = np.eye(128).astype(mld.bfloat16)
    p_i = np.arange(128)[:, None]
    f_i = np.arange(128)[None, :]
    maskd = (f_i - p_i >= 0).astype(mld.bfloat16)

    in_maps = []
    for c in range(NCORES):
        wq_c = wq[c * 512:(c + 1) * 512, :]          # [512, HID]
        wk_c = wk[c * 128:(c + 1) * 128, :]
        wv_c = wv[c * 128:(c + 1) * 128, :]
        wo_c = wo[:, c * 512:(c + 1) * 512]          # [HID, 512]

        # wq lhsT head-major: [QH, ND, 128, 128]
        wq_t = np.ascontiguousarray(
            wq_c.T.reshape(ND, 128, QH, 128).transpose(2, 0, 1, 3)
        )
        wq8 = pack_lhs(*split8(wq_t * WS))           # [128, QH, ND, 2, 128]
        wk8 = pack_lhs(*split8(wk_c.T.reshape(ND, 128, 128) * WS))
        wv8 = pack_lhs(*split8(wv_c.T.reshape(ND, 128, 128) * WS))
        # wo rhs: per head [QH, 128 hd, HID], slots (wo2, wo1)
        wo_t = np.ascontiguousarray(wo_c.T.reshape(QH, 128, HID))
        wo8 = pack_rhs(*split8(wo_t * WS))           # [128, QH, 2, HID]

        in_maps.append({
            "ht": ht8,
            "wq": wq8,
            "wk": wk8,
            "wv": wv8,
            "wo": wo8,
            "cosd": cosd,
            "sind": sind,
            "identd": iden16,
            "identb": idenbf,
            "maskd": maskd,
        })
    return in_maps


def kernel(h, position_ids, wq, wk, wv, wo):
    global LAST_RESULTS
    from concourse.bass_utils import run_bass_kernel_spmd

    if "nc" not in _CACHE:
        _CACHE["nc"] = _build_program()
    nc = _CACHE["nc"]

    in_maps = _pack_inputs(
        np.asarray(h, dtype=np.float32),
        np.asarray(position_ids),
        np.asarray(wq, dtype=np.float32),
        np.asarray(wk, dtype=np.float32),
        np.asarray(wv, dtype=np.float32),
        np.asarray(wo, dtype=np.float32),
    )

    trace = bool(int(os.environ.get("KERNEL_TRACE", "0")))
    res = run_bass_kernel_spmd(
        nc, in_maps, core_ids=list(range(NCORES)), trace=trace
    )
    LAST_RESULTS = res

    acc = np.zeros((S, HID), dtype=np.float32)
    for r in res.results:
        acc += r["out"].astype(np.float32)
    return acc
